# revision 15
# baseline (speedup 1.0000x reference)
import sys

sys.path.insert(0, "/opt/trn_rl_repo")

import numpy as np
import ml_dtypes

import concourse.bass as bass
from concourse import bacc
import concourse.mybir as mybir
import concourse.tile as tile
from concourse.bass_utils import run_bass_kernel_spmd

# Problem constants (nn_ConvLSTMAutoencoder: B=128, T=100, F=64, hid [16,32])
B_TOT, T, F = 128, 100, 64
NCORES = 8
B = B_TOT // NCORES          # 16 batch per core (pure data parallelism)
SEG = F + 2                  # spatial row with 1 zero pad col each side
C0, C1 = 16, 32

F32 = mybir.dt.float32
BF16 = mybir.dt.bfloat16
NP_BF16 = ml_dtypes.bfloat16

ACT_DT = BF16                # arena / gate tensors / matmul inputs
C_DT = BF16                  # cell-state dtype (flip to F32 if accuracy needs)

Tanh = mybir.ActivationFunctionType.Tanh
Sigmoid = mybir.ActivationFunctionType.Sigmoid
MULT = mybir.AluOpType.mult
ADD = mybir.AluOpType.add

NB = 8                       # batches per matmul (8*64 = 512 = psum bank cap)
NDUM = 8                     # PE-warming dummy matmuls per timestep

# Gate column spread along matmul M for every layer:
#   [i @ 0:C | f @ 32:32+C | o @ 64:64+C | g @ 96:96+C]
# Cell-state tiles keep c at rows 32:32+C (v/add operate at base 32).
# tanh(c) is written at rows 64:64+C to meet sigma(o) for the h product.


def _taps(nc, zt, wt, rhs_rows, arena):
    """3-tap conv along F as PSUM-accumulated matmuls, nb-major so each
    batch-half's z finishes as early as possible. wt: [K, 3, M]."""
    for nb in range(0, B, NB):
        for d in range(3):
            nc.tensor.matmul(
                zt[:, nb:nb + NB, :],
                wt[:, d, :],
                arena[rhs_rows, nb:nb + NB, d:d + F],
                start=(d == 0),
                stop=(d == 2),
            )


def _cell_sig(nc, wpool, z, C, bvec, ctile, h_of):
    """LSTM cell, act-heavy variant (true Sigmoid + Tanh acts), processed in
    independent batch-halves so the recurrence chain uses half-size ops.

    Emission order keeps each engine queue free of head-of-line blocking:
    all z-activations first, then the DVE c-updates, then tanh(c) acts,
    then the h products."""
    s = wpool.tile([96, B, F], ACT_DT, tag="s", name="s")
    tg = wpool.tile([C, B, F], ACT_DT, tag="tg", name="tg")
    u = wpool.tile([32 + C, B, F], ACT_DT, tag="u", name="u")
    v = wpool.tile([32 + C, B, F], C_DT, tag="v", name="v")
    tc = wpool.tile([64 + C, B, F], ACT_DT, tag="tc", name="tc")
    halves = [(n0, n0 + NB) for n0 in range(0, B, NB)]
    for n0, n1 in halves:
        nc.scalar.activation(tg[:, n0:n1], z[96:96 + C, n0:n1], Tanh,
                             bias=bvec[96:96 + C])
        nc.scalar.activation(s[0:64 + C, n0:n1], z[0:64 + C, n0:n1], Sigmoid,
                             bias=bvec[0:64 + C])
        nc.vector.tensor_tensor(v[32:32 + C, n0:n1], s[32:32 + C, n0:n1],
                                ctile[32:32 + C, n0:n1], MULT)
        nc.vector.tensor_tensor(u[32:32 + C, n0:n1], s[0:C, n0:n1],
                                tg[:, n0:n1], MULT)
        nc.vector.tensor_tensor(ctile[32:32 + C, n0:n1], u[32:32 + C, n0:n1],
                                v[32:32 + C, n0:n1], ADD)
    for n0, n1 in halves:
        nc.scalar.activation(tc[64:64 + C, n0:n1], ctile[32:32 + C, n0:n1],
                             Tanh)
    for n0, n1 in halves:
        nc.vector.tensor_tensor(h_of(n0, n1), s[64:64 + C, n0:n1],
                                tc[64:64 + C, n0:n1], MULT)


def _cell_tanh(nc, wpool, z, C, M, bvec, ctile, h_of):
    """LSTM cell, DVE-heavy variant: one Tanh act (i/f/o pre-halved in the
    weights), sigma fixup + g re-base on the vector engine. Batch-halved."""
    th = wpool.tile([M, B, F], ACT_DT, tag="s", name="th")
    tg = wpool.tile([C, B, F], ACT_DT, tag="tg", name="tg")
    u = wpool.tile([32 + C, B, F], ACT_DT, tag="u", name="u")
    v = wpool.tile([32 + C, B, F], C_DT, tag="v", name="v")
    tc = wpool.tile([64 + C, B, F], ACT_DT, tag="tc", name="tc")
    halves = [(n0, n0 + NB) for n0 in range(0, B, NB)]
    for n0, n1 in halves:
        nc.scalar.activation(th[:, n0:n1], z[0:M, n0:n1], Tanh, bias=bvec[0:M])
        nc.vector.tensor_scalar(tg[:, n0:n1], th[96:96 + C, n0:n1],
                                1.0, 0.0, MULT, ADD)
        nc.vector.tensor_scalar(th[0:64 + C, n0:n1], th[0:64 + C, n0:n1],
                                0.5, 0.5, MULT, ADD)
        nc.vector.tensor_tensor(v[32:32 + C, n0:n1], th[32:32 + C, n0:n1],
                                ctile[32:32 + C, n0:n1], MULT)
        nc.vector.tensor_tensor(u[32:32 + C, n0:n1], th[0:C, n0:n1],
                                tg[:, n0:n1], MULT)
        nc.vector.tensor_tensor(ctile[32:32 + C, n0:n1], u[32:32 + C, n0:n1],
                                v[32:32 + C, n0:n1], ADD)
    for n0, n1 in halves:
        nc.scalar.activation(tc[64:64 + C, n0:n1], ctile[32:32 + C, n0:n1],
                             Tanh)
    for n0, n1 in halves:
        nc.vector.tensor_tensor(h_of(n0, n1), th[64:64 + C, n0:n1],
                                tc[64:64 + C, n0:n1], MULT)


def _dummy_mms(nc, dpool, wsrc, n):
    """Keep the PE array continuously busy between real matmul bursts so it
    stays at the high DVFS pstate. Reads only constant weight tiles."""
    for _ in range(n):
        dz = dpool.tile([128, 384], F32, tag="dz", name="dz")
        nc.tensor.matmul(dz[:], wsrc[0:64, 0, :], wsrc[0:64, :, :],
                         start=True, stop=True)


def build_program():
    nc = bacc.Bacc(None)

    x_pad = nc.declare_dram_parameter("x_pad", [T, B, SEG], ACT_DT, isOutput=False)
    # enc0 lhsT variants A/B: K=18 rows [h0(16), xA, xB]
    we0a = nc.declare_dram_parameter("we0a", [18, 3, 112], ACT_DT, isOutput=False)
    we0b = nc.declare_dram_parameter("we0b", [18, 3, 112], ACT_DT, isOutput=False)
    we1 = nc.declare_dram_parameter("we1", [64, 3, 128], ACT_DT, isOutput=False)
    wd0a = nc.declare_dram_parameter("wd0a", [128, 3, 128], ACT_DT, isOutput=False)
    wd0b = nc.declare_dram_parameter("wd0b", [128, 3, 128], ACT_DT, isOutput=False)
    wd1a = nc.declare_dram_parameter("wd1a", [128, 3, 112], ACT_DT, isOutput=False)
    wd1b = nc.declare_dram_parameter("wd1b", [128, 3, 112], ACT_DT, isOutput=False)
    fcv = nc.declare_dram_parameter("fcv", [17, 1], ACT_DT, isOutput=False)
    b0 = nc.declare_dram_parameter("b0", [112, 1], F32, isOutput=False)
    b1 = nc.declare_dram_parameter("b1", [128, 1], F32, isOutput=False)
    bd0 = nc.declare_dram_parameter("bd0", [128, 1], F32, isOutput=False)
    bd1 = nc.declare_dram_parameter("bd1", [112, 1], F32, isOutput=False)
    out = nc.declare_dram_parameter("out", [B, T, F], F32, isOutput=True)

    with tile.TileContext(nc) as tc:
        with (
            tc.tile_pool(name="const", bufs=1) as cpool,
            tc.tile_pool(name="state", bufs=1) as spool,
            tc.tile_pool(name="work", bufs=2) as wpool,
            tc.tile_pool(name="zp", bufs=2, space="PSUM") as zpool,
            tc.tile_pool(name="fcp", bufs=1, space="PSUM") as fcpool,
            tc.tile_pool(name="dum", bufs=1, space="PSUM") as dpool,
        ):
            we0at = cpool.tile([18, 3, 112], ACT_DT)
            we0bt = cpool.tile([18, 3, 112], ACT_DT)
            we1t = cpool.tile([64, 3, 128], ACT_DT)
            wd0at = cpool.tile([128, 3, 128], ACT_DT)
            wd0bt = cpool.tile([128, 3, 128], ACT_DT)
            wd1at = cpool.tile([128, 3, 112], ACT_DT)
            wd1bt = cpool.tile([128, 3, 112], ACT_DT)
            fcvt = cpool.tile([17, 1], ACT_DT)
            b0t = cpool.tile([112, 1], F32)
            b1t = cpool.tile([128, 1], F32)
            bd0t = cpool.tile([128, 1], F32)
            bd1t = cpool.tile([112, 1], F32)
            for dst, dsrc in [(we0at, we0a), (we0bt, we0b), (we1t, we1),
                              (wd0at, wd0a), (wd0bt, wd0b), (wd1at, wd1a),
                              (wd1bt, wd1b), (fcvt, fcv), (b0t, b0),
                              (b1t, b1), (bd0t, bd0), (bd1t, bd1)]:
                nc.sync.dma_start(dst[:], dsrc[:])

            # Encoder arena rows: 0:16 h0 | 16 xA | 17 xB | 18:32 junk | 32:64 h1
            # Decoder arena rows: 0:16 hd1 | 16 ones | 17:32 junk |
            #                     32:64 e2A | 64:96 hd0 | 96:128 e2B
            arena_e = spool.tile([64, B, SEG], ACT_DT)
            arena_d = spool.tile([128, B, SEG], ACT_DT)
            seq = spool.tile([128, (T + 3) // 4, B, F], ACT_DT)
            nc.vector.memset(arena_e[:], 0.0)
            nc.vector.memset(arena_d[:], 0.0)
            nc.vector.memset(arena_d[0:17], 1.0)   # row 16 stays 1.0 (fc bias)
            nc.vector.memset(arena_d[0:16], 0.0)

            ce0 = spool.tile([32 + C0, B, F], C_DT)
            ce1 = spool.tile([32 + C1, B, F], C_DT)
            cd0 = spool.tile([32 + C1, B, F], C_DT)
            cd1 = spool.tile([32 + C0, B, F], C_DT)
            nc.vector.memset(ce0[32:32 + C0], 0.0)
            nc.vector.memset(ce1[32:32 + C1], 0.0)

            # ---------------- encoder ----------------
            nc.sync.dma_start(arena_e[16:17, :, :], x_pad[0:1, :, :])
            for t in range(T):
                if t + 1 < T:
                    xrow = 16 + ((t + 1) % 2)
                    nc.sync.dma_start(arena_e[xrow:xrow + 1, :, :],
                                      x_pad[t + 1:t + 2, :, :])

                z0 = zpool.tile([112, B, F], F32, tag="z", name="z0")
                _taps(nc, z0, we0at if t % 2 == 0 else we0bt,
                      slice(0, 18), arena_e)
                _cell_sig(nc, wpool, z0, C0, b0t, ce0,
                          lambda n0, n1: arena_e[0:C0, n0:n1, 1:1 + F])

                z1 = zpool.tile([128, B, F], F32, tag="z", name="z1")
                _taps(nc, z1, we1t, slice(0, 64), arena_e)
                _dummy_mms(nc, dpool, wd0at, NDUM)
                _cell_tanh(nc, wpool, z1, C1, 128, b1t, ce1,
                           lambda n0, n1: arena_e[C1:2 * C1, n0:n1, 1:1 + F])

                r = (t % 4) * 32
                nc.sync.dma_start(seq[r:r + 32, t // 4],
                                  arena_e[C1:2 * C1, :, 1:1 + F])

            # ---------------- decoder init ----------------
            nc.sync.dma_start(arena_d[0:16, :, :], arena_e[0:16, :, :])
            nc.sync.dma_start(arena_d[64:96, :, :], arena_e[32:64, :, :])
            nc.sync.dma_start(cd0[32:32 + C1], ce1[32:32 + C1])
            nc.sync.dma_start(cd1[32:32 + C0], ce0[32:32 + C0])
            nc.sync.dma_start(arena_d[32:64, :, 1:1 + F], seq[0:32, 0])

            # ---------------- decoder ----------------
            # fc for step t is emitted at the top of iteration t+1 (hd1(t)
            # stays valid in the arena until dec1's h write late in t+1), so
            # its matmuls never block the next step's taps.
            def emit_fc(t):
                zfc = fcpool.tile([1, B, F], F32, tag="fc", name="zfc")
                for nb in range(0, B, NB):
                    nc.tensor.matmul(zfc[:, nb:nb + NB, :], fcvt[:],
                                     arena_d[0:17, nb:nb + NB, 1:1 + F],
                                     start=True, stop=True)
                ofc = wpool.tile([1, B, F], F32, tag="ofc", name="ofc")
                nc.scalar.copy(ofc[:], zfc[:])
                nc.sync.dma_start(out[:, t, :], ofc[0:1, :, :])

            for t in range(T):
                if t > 0:
                    emit_fc(t - 1)
                if t + 1 < T:
                    r = ((t + 1) % 4) * 32
                    e2rows = slice(32, 64) if (t + 1) % 2 == 0 else slice(96, 128)
                    nc.sync.dma_start(arena_d[e2rows, :, 1:1 + F],
                                      seq[r:r + 32, (t + 1) // 4])

                zd0 = zpool.tile([128, B, F], F32, tag="z", name="zd0")
                _taps(nc, zd0, wd0at if t % 2 == 0 else wd0bt,
                      slice(0, 128), arena_d)
                _cell_tanh(nc, wpool, zd0, C1, 128, bd0t, cd0,
                           lambda n0, n1: arena_d[64:96, n0:n1, 1:1 + F])

                zd1 = zpool.tile([112, B, F], F32, tag="z", name="zd1")
                _taps(nc, zd1, wd1at if t % 2 == 0 else wd1bt,
                      slice(0, 128), arena_d)
                _dummy_mms(nc, dpool, wd0at, NDUM)
                _cell_sig(nc, wpool, zd1, C0, bd1t, cd1,
                          lambda n0, n1: arena_d[0:C0, n0:n1, 1:1 + F])

            emit_fc(T - 1)

    nc.finalize()
    return nc


def _prep_weights(w, b, Cin, C, row_map, halve_ifo, M):
    """[4C, Cin, 3, 3] -> lhsT [len(row_map), 3, M], bias [M, 1].

    Gate order i,f,o,g -> M columns i@0 f@32 o@64 g@96. row_map maps lhsT
    row -> input channel (-1 = zero row). halve_ifo scales i/f/o rows by
    0.5 for the tanh-trick cells.
    """
    w3 = np.asarray(w, np.float32).reshape(4 * C, Cin, 3, 3)[:, :, :, 1]
    b = np.asarray(b, np.float32).reshape(4 * C)
    lhsT = np.zeros((len(row_map), 3, M), np.float32)
    bvec = np.zeros((M, 1), np.float32)
    for gi, col0 in enumerate((0, 32, 64, 96)):
        scale = 0.5 if (halve_ifo and gi != 3) else 1.0
        for j in range(C):
            oc = gi * C + j
            bvec[col0 + j, 0] = b[oc] * scale
            for r, ch in enumerate(row_map):
                if ch >= 0:
                    lhsT[r, :, col0 + j] = w3[oc, ch, :] * scale
    return np.ascontiguousarray(lhsT).astype(NP_BF16), bvec


_CACHE = {}


def kernel(x, enc_w0, enc_b0, enc_w1, enc_b1, dec_w0, dec_b0, dec_w1, dec_b1,
           fc_w, fc_b):
    if "nc" not in _CACHE:
        _CACHE["nc"] = build_program()
    nc = _CACHE["nc"]

    x = np.asarray(x, np.float32)
    ZR = [-1]
    # enc0: input ch = [x(0), h0(1:17)]; arena rows [h0(16), xA, xB]
    we0a, b0 = _prep_weights(enc_w0, enc_b0, 1 + C0, C0,
                             list(range(1, 17)) + [0] + ZR, False, 112)
    we0b, _ = _prep_weights(enc_w0, enc_b0, 1 + C0, C0,
                            list(range(1, 17)) + ZR + [0], False, 112)
    # enc1: input ch = [h0(0:16), h1(16:48)]; rows [h0, xA, xB, junk*14, h1]
    we1, b1 = _prep_weights(enc_w1, enc_b1, C0 + C1, C1,
                            list(range(16)) + ZR * 16 + list(range(16, 48)),
                            True, 128)
    # dec0: input ch = [e2(0:32), hd0(32:64)];
    # rows [hd1+ones+junk(0:32), e2A(32:64), hd0(64:96), e2B(96:128)]
    wd0a, bd0 = _prep_weights(dec_w0, dec_b0, C1 + C1, C1,
                              ZR * 32 + list(range(32)) +
                              list(range(32, 64)) + ZR * 32, True, 128)
    wd0b, _ = _prep_weights(dec_w0, dec_b0, C1 + C1, C1,
                            ZR * 32 + ZR * 32 +
                            list(range(32, 64)) + list(range(32)), True, 128)
    # dec1: input ch = [hd0(0:32), hd1(32:48)] — input is dec0's output seq
    wd1a, bd1 = _prep_weights(dec_w1, dec_b1, C1 + C0, C0,
                              list(range(32, 48)) + ZR * 16 + ZR * 32 +
                              list(range(32)) + ZR * 32, False, 112)
    wd1b = wd1a
    fcv = np.concatenate(
        [np.asarray(fc_w, np.float32).reshape(C0),
         np.asarray(fc_b, np.float32).reshape(1)]).reshape(17, 1)
    fcv = np.ascontiguousarray(fcv).astype(NP_BF16)

    in_maps = []
    for core in range(NCORES):
        xs = x[core * B:(core + 1) * B]      # [B, T, F]
        xp = np.zeros((T, B, SEG), np.float32)
        xp[:, :, 1:1 + F] = xs.transpose(1, 0, 2)
        in_maps.append({
            "x_pad": xp.astype(NP_BF16),
            "we0a": we0a, "we0b": we0b, "we1": we1,
            "wd0a": wd0a, "wd0b": wd0b, "wd1a": wd1a, "wd1b": wd1b,
            "fcv": fcv,
            "b0": b0, "b1": b1, "bd0": bd0, "bd1": bd1,
        })

    _CACHE["in_maps"] = in_maps
    res = run_bass_kernel_spmd(nc, in_maps, core_ids=list(range(NCORES)))
    outs = [res.results[i]["out"] for i in range(NCORES)]
    return np.concatenate(outs, axis=0).astype(np.float32)


if __name__ == "__main__":
    rng = np.random.default_rng(0)
    inputs = {
        "x": rng.standard_normal((B_TOT, T, F), dtype=np.float32),
        "enc_w0": rng.standard_normal((4 * C0, 1 + C0, 3, 3), dtype=np.float32) * 0.05,
        "enc_b0": np.zeros(4 * C0, np.float32),
        "enc_w1": rng.standard_normal((4 * C1, C0 + C1, 3, 3), dtype=np.float32) * 0.05,
        "enc_b1": np.zeros(4 * C1, np.float32),
        "dec_w0": rng.standard_normal((4 * C1, C1 + C1, 3, 3), dtype=np.float32) * 0.05,
        "dec_b0": np.zeros(4 * C1, np.float32),
        "dec_w1": rng.standard_normal((4 * C0, C1 + C0, 3, 3), dtype=np.float32) * 0.05,
        "dec_b1": np.zeros(4 * C0, np.float32),
        "fc_w": rng.standard_normal((1, C0, 1, 1), dtype=np.float32) * 0.05,
        "fc_b": np.zeros(1, np.float32),
    }
    out = kernel(**inputs)
    print("out", out.shape, out.dtype, np.abs(out).max())


# revision 16
# speedup vs baseline: 1.1575x; 1.1575x over previous
import sys

sys.path.insert(0, "/opt/trn_rl_repo")

import numpy as np
import ml_dtypes

import concourse.bass as bass
from concourse import bacc
import concourse.mybir as mybir
import concourse.tile as tile
from concourse.bass_utils import run_bass_kernel_spmd

# Problem constants (nn_ConvLSTMAutoencoder: B=128, T=100, F=64, hid [16,32])
B_TOT, T, F = 128, 100, 64
NCORES = 8
B = B_TOT // NCORES          # 16 batch per core (pure data parallelism)
SEG = F + 2                  # spatial row with 1 zero pad col each side
C0, C1 = 16, 32

F32 = mybir.dt.float32
BF16 = mybir.dt.bfloat16
NP_BF16 = ml_dtypes.bfloat16

ACT_DT = BF16                # arena / gate tensors / matmul inputs
C_DT = BF16                  # cell-state dtype (flip to F32 if accuracy needs)

Tanh = mybir.ActivationFunctionType.Tanh
Sigmoid = mybir.ActivationFunctionType.Sigmoid
MULT = mybir.AluOpType.mult
ADD = mybir.AluOpType.add

NB = 8                       # batches per matmul (8*64 = 512 = psum bank cap)

# Gate column spread along matmul M for every layer:
#   [i @ 0:C | f @ 32:32+C | o @ 64:64+C | g @ 96:96+C]
# Cell-state tiles keep c at rows 32:32+C (v/add operate at base 32).
# tanh(c) is written at rows 64:64+C to meet sigma(o) for the h product.


def _taps(nc, zt, wt, rhs_rows, arena):
    """3-tap conv along F as PSUM-accumulated matmuls, nb-major so each
    batch-half's z finishes as early as possible. wt: [K, 3, M]."""
    for nb in range(0, B, NB):
        for d in range(3):
            nc.tensor.matmul(
                zt[:, nb:nb + NB, :],
                wt[:, d, :],
                arena[rhs_rows, nb:nb + NB, d:d + F],
                start=(d == 0),
                stop=(d == 2),
            )


def _cell_sig(nc, wpool, z, C, bvec, ctile, h_of):
    """LSTM cell, act-heavy variant (true Sigmoid + Tanh acts), processed in
    independent batch-halves so the recurrence chain uses half-size ops.

    Emission order keeps each engine queue free of head-of-line blocking:
    all z-activations first, then the DVE c-updates, then tanh(c) acts,
    then the h products."""
    s = wpool.tile([96, B, F], ACT_DT, tag="s", name="s")
    tg = wpool.tile([C, B, F], ACT_DT, tag="tg", name="tg")
    u = wpool.tile([32 + C, B, F], ACT_DT, tag="u", name="u")
    v = wpool.tile([32 + C, B, F], C_DT, tag="v", name="v")
    tc = wpool.tile([64 + C, B, F], ACT_DT, tag="tc", name="tc")
    halves = [(n0, n0 + NB) for n0 in range(0, B, NB)]
    for n0, n1 in halves:
        nc.scalar.activation(tg[:, n0:n1], z[96:96 + C, n0:n1], Tanh,
                             bias=bvec[96:96 + C])
        nc.scalar.activation(s[0:64 + C, n0:n1], z[0:64 + C, n0:n1], Sigmoid,
                             bias=bvec[0:64 + C])
        nc.vector.tensor_tensor(v[32:32 + C, n0:n1], s[32:32 + C, n0:n1],
                                ctile[32:32 + C, n0:n1], MULT)
        nc.vector.tensor_tensor(u[32:32 + C, n0:n1], s[0:C, n0:n1],
                                tg[:, n0:n1], MULT)
        nc.vector.tensor_tensor(ctile[32:32 + C, n0:n1], u[32:32 + C, n0:n1],
                                v[32:32 + C, n0:n1], ADD)
    for n0, n1 in halves:
        nc.scalar.activation(tc[64:64 + C, n0:n1], ctile[32:32 + C, n0:n1],
                             Tanh)
    for n0, n1 in halves:
        nc.vector.tensor_tensor(h_of(n0, n1), s[64:64 + C, n0:n1],
                                tc[64:64 + C, n0:n1], MULT)


def _cell_tanh(nc, wpool, z, C, M, bvec, ctile, h_of):
    """LSTM cell, DVE-heavy variant: one Tanh act (i/f/o pre-halved in the
    weights), sigma fixup + g re-base on the vector engine. Batch-halved."""
    th = wpool.tile([M, B, F], ACT_DT, tag="s", name="th")
    tg = wpool.tile([C, B, F], ACT_DT, tag="tg", name="tg")
    u = wpool.tile([32 + C, B, F], ACT_DT, tag="u", name="u")
    v = wpool.tile([32 + C, B, F], C_DT, tag="v", name="v")
    tc = wpool.tile([64 + C, B, F], ACT_DT, tag="tc", name="tc")
    halves = [(n0, n0 + NB) for n0 in range(0, B, NB)]
    for n0, n1 in halves:
        nc.scalar.activation(th[:, n0:n1], z[0:M, n0:n1], Tanh, bias=bvec[0:M])
        nc.vector.tensor_scalar(tg[:, n0:n1], th[96:96 + C, n0:n1],
                                1.0, 0.0, MULT, ADD)
        nc.vector.tensor_scalar(th[0:64 + C, n0:n1], th[0:64 + C, n0:n1],
                                0.5, 0.5, MULT, ADD)
        nc.vector.tensor_tensor(v[32:32 + C, n0:n1], th[32:32 + C, n0:n1],
                                ctile[32:32 + C, n0:n1], MULT)
        nc.vector.tensor_tensor(u[32:32 + C, n0:n1], th[0:C, n0:n1],
                                tg[:, n0:n1], MULT)
        nc.vector.tensor_tensor(ctile[32:32 + C, n0:n1], u[32:32 + C, n0:n1],
                                v[32:32 + C, n0:n1], ADD)
    for n0, n1 in halves:
        nc.scalar.activation(tc[64:64 + C, n0:n1], ctile[32:32 + C, n0:n1],
                             Tanh)
    for n0, n1 in halves:
        nc.vector.tensor_tensor(h_of(n0, n1), th[64:64 + C, n0:n1],
                                tc[64:64 + C, n0:n1], MULT)


def _dummy_mms(nc, dpool, wsrc, n):
    """Keep the PE array continuously busy between real matmul bursts so it
    stays at the high DVFS pstate. Reads only constant weight tiles."""
    for _ in range(n):
        dz = dpool.tile([128, 384], F32, tag="dz", name="dz")
        nc.tensor.matmul(dz[:], wsrc[0:64, 0, :], wsrc[0:64, :, :],
                         start=True, stop=True)


def build_program():
    nc = bacc.Bacc(None)

    x_pad = nc.declare_dram_parameter("x_pad", [T, B, SEG], ACT_DT, isOutput=False)
    # enc0 lhsT variants A/B: K=18 rows [h0(16), xA, xB]
    we0a = nc.declare_dram_parameter("we0a", [18, 3, 112], ACT_DT, isOutput=False)
    we0b = nc.declare_dram_parameter("we0b", [18, 3, 112], ACT_DT, isOutput=False)
    we1 = nc.declare_dram_parameter("we1", [64, 3, 128], ACT_DT, isOutput=False)
    wd0a = nc.declare_dram_parameter("wd0a", [128, 3, 128], ACT_DT, isOutput=False)
    wd0b = nc.declare_dram_parameter("wd0b", [128, 3, 128], ACT_DT, isOutput=False)
    wd1a = nc.declare_dram_parameter("wd1a", [128, 3, 112], ACT_DT, isOutput=False)
    wd1b = nc.declare_dram_parameter("wd1b", [128, 3, 112], ACT_DT, isOutput=False)
    fcv = nc.declare_dram_parameter("fcv", [17, 1], ACT_DT, isOutput=False)
    b0 = nc.declare_dram_parameter("b0", [112, 1], F32, isOutput=False)
    b1 = nc.declare_dram_parameter("b1", [128, 1], F32, isOutput=False)
    bd0 = nc.declare_dram_parameter("bd0", [128, 1], F32, isOutput=False)
    bd1 = nc.declare_dram_parameter("bd1", [112, 1], F32, isOutput=False)
    out = nc.declare_dram_parameter("out", [B, T, F], F32, isOutput=True)

    with tile.TileContext(nc) as tc:
        with (
            tc.tile_pool(name="const", bufs=1) as cpool,
            tc.tile_pool(name="state", bufs=1) as spool,
            tc.tile_pool(name="work", bufs=2) as wpool,
            tc.tile_pool(name="zp", bufs=2, space="PSUM") as zpool,
            tc.tile_pool(name="fcp", bufs=1, space="PSUM") as fcpool,
        ):
            we0at = cpool.tile([18, 3, 112], ACT_DT)
            we0bt = cpool.tile([18, 3, 112], ACT_DT)
            we1t = cpool.tile([64, 3, 128], ACT_DT)
            wd0at = cpool.tile([128, 3, 128], ACT_DT)
            wd0bt = cpool.tile([128, 3, 128], ACT_DT)
            wd1at = cpool.tile([128, 3, 112], ACT_DT)
            wd1bt = cpool.tile([128, 3, 112], ACT_DT)
            fcvt = cpool.tile([17, 1], ACT_DT)
            b0t = cpool.tile([112, 1], F32)
            b1t = cpool.tile([128, 1], F32)
            bd0t = cpool.tile([128, 1], F32)
            bd1t = cpool.tile([112, 1], F32)
            for dst, dsrc in [(we0at, we0a), (we0bt, we0b), (we1t, we1),
                              (wd0at, wd0a), (wd0bt, wd0b), (wd1at, wd1a),
                              (wd1bt, wd1b), (fcvt, fcv), (b0t, b0),
                              (b1t, b1), (bd0t, bd0), (bd1t, bd1)]:
                nc.sync.dma_start(dst[:], dsrc[:])

            # Encoder arena rows: 0:16 h0 | 16 xA | 17 xB | 18:32 junk | 32:64 h1
            # Decoder arena rows: 0:16 hd1 | 16 ones | 17:32 junk |
            #                     32:64 e2A | 64:96 hd0 | 96:128 e2B
            arena_e = spool.tile([64, B, SEG], ACT_DT)
            arena_d = spool.tile([128, B, SEG], ACT_DT)
            seq = spool.tile([128, (T + 3) // 4, B, F], ACT_DT)
            nc.vector.memset(arena_e[:], 0.0)
            nc.vector.memset(arena_d[:], 0.0)
            nc.vector.memset(arena_d[0:17], 1.0)   # row 16 stays 1.0 (fc bias)
            nc.vector.memset(arena_d[0:16], 0.0)

            ce0 = spool.tile([32 + C0, B, F], C_DT)
            ce1 = spool.tile([32 + C1, B, F], C_DT)
            cd0 = spool.tile([32 + C1, B, F], C_DT)
            cd1 = spool.tile([32 + C0, B, F], C_DT)
            nc.vector.memset(ce0[32:32 + C0], 0.0)
            nc.vector.memset(ce1[32:32 + C1], 0.0)

            # ---------------- encoder ----------------
            nc.sync.dma_start(arena_e[16:17, :, :], x_pad[0:1, :, :])
            for t in range(T):
                if t + 1 < T:
                    xrow = 16 + ((t + 1) % 2)
                    nc.sync.dma_start(arena_e[xrow:xrow + 1, :, :],
                                      x_pad[t + 1:t + 2, :, :])

                z0 = zpool.tile([112, B, F], F32, tag="z", name="z0")
                _taps(nc, z0, we0at if t % 2 == 0 else we0bt,
                      slice(0, 18), arena_e)
                _cell_sig(nc, wpool, z0, C0, b0t, ce0,
                          lambda n0, n1: arena_e[0:C0, n0:n1, 1:1 + F])

                z1 = zpool.tile([128, B, F], F32, tag="z", name="z1")
                _taps(nc, z1, we1t, slice(0, 64), arena_e)
                _cell_tanh(nc, wpool, z1, C1, 128, b1t, ce1,
                           lambda n0, n1: arena_e[C1:2 * C1, n0:n1, 1:1 + F])

                r = (t % 4) * 32
                nc.sync.dma_start(seq[r:r + 32, t // 4],
                                  arena_e[C1:2 * C1, :, 1:1 + F])

            # ---------------- decoder init ----------------
            nc.sync.dma_start(arena_d[0:16, :, :], arena_e[0:16, :, :])
            nc.sync.dma_start(arena_d[64:96, :, :], arena_e[32:64, :, :])
            nc.sync.dma_start(cd0[32:32 + C1], ce1[32:32 + C1])
            nc.sync.dma_start(cd1[32:32 + C0], ce0[32:32 + C0])
            nc.sync.dma_start(arena_d[32:64, :, 1:1 + F], seq[0:32, 0])

            # ---------------- decoder ----------------
            # fc for step t is emitted at the top of iteration t+1 (hd1(t)
            # stays valid in the arena until dec1's h write late in t+1), so
            # its matmuls never block the next step's taps.
            def emit_fc(t):
                zfc = fcpool.tile([1, B, F], F32, tag="fc", name="zfc")
                for nb in range(0, B, NB):
                    nc.tensor.matmul(zfc[:, nb:nb + NB, :], fcvt[:],
                                     arena_d[0:17, nb:nb + NB, 1:1 + F],
                                     start=True, stop=True)
                ofc = wpool.tile([1, B, F], F32, tag="ofc", name="ofc")
                nc.scalar.copy(ofc[:], zfc[:])
                nc.sync.dma_start(out[:, t, :], ofc[0:1, :, :])

            for t in range(T):
                if t > 0:
                    emit_fc(t - 1)
                if t + 1 < T:
                    r = ((t + 1) % 4) * 32
                    e2rows = slice(32, 64) if (t + 1) % 2 == 0 else slice(96, 128)
                    nc.sync.dma_start(arena_d[e2rows, :, 1:1 + F],
                                      seq[r:r + 32, (t + 1) // 4])

                zd0 = zpool.tile([128, B, F], F32, tag="z", name="zd0")
                _taps(nc, zd0, wd0at if t % 2 == 0 else wd0bt,
                      slice(0, 128), arena_d)
                _cell_tanh(nc, wpool, zd0, C1, 128, bd0t, cd0,
                           lambda n0, n1: arena_d[64:96, n0:n1, 1:1 + F])

                zd1 = zpool.tile([112, B, F], F32, tag="z", name="zd1")
                _taps(nc, zd1, wd1at if t % 2 == 0 else wd1bt,
                      slice(0, 128), arena_d)
                _cell_sig(nc, wpool, zd1, C0, bd1t, cd1,
                          lambda n0, n1: arena_d[0:C0, n0:n1, 1:1 + F])

            emit_fc(T - 1)

    nc.finalize()
    return nc


def _prep_weights(w, b, Cin, C, row_map, halve_ifo, M):
    """[4C, Cin, 3, 3] -> lhsT [len(row_map), 3, M], bias [M, 1].

    Gate order i,f,o,g -> M columns i@0 f@32 o@64 g@96. row_map maps lhsT
    row -> input channel (-1 = zero row). halve_ifo scales i/f/o rows by
    0.5 for the tanh-trick cells.
    """
    w3 = np.asarray(w, np.float32).reshape(4 * C, Cin, 3, 3)[:, :, :, 1]
    b = np.asarray(b, np.float32).reshape(4 * C)
    lhsT = np.zeros((len(row_map), 3, M), np.float32)
    bvec = np.zeros((M, 1), np.float32)
    for gi, col0 in enumerate((0, 32, 64, 96)):
        scale = 0.5 if (halve_ifo and gi != 3) else 1.0
        for j in range(C):
            oc = gi * C + j
            bvec[col0 + j, 0] = b[oc] * scale
            for r, ch in enumerate(row_map):
                if ch >= 0:
                    lhsT[r, :, col0 + j] = w3[oc, ch, :] * scale
    return np.ascontiguousarray(lhsT).astype(NP_BF16), bvec


_CACHE = {}


def kernel(x, enc_w0, enc_b0, enc_w1, enc_b1, dec_w0, dec_b0, dec_w1, dec_b1,
           fc_w, fc_b):
    if "nc" not in _CACHE:
        _CACHE["nc"] = build_program()
    nc = _CACHE["nc"]

    x = np.asarray(x, np.float32)
    ZR = [-1]
    # enc0: input ch = [x(0), h0(1:17)]; arena rows [h0(16), xA, xB]
    we0a, b0 = _prep_weights(enc_w0, enc_b0, 1 + C0, C0,
                             list(range(1, 17)) + [0] + ZR, False, 112)
    we0b, _ = _prep_weights(enc_w0, enc_b0, 1 + C0, C0,
                            list(range(1, 17)) + ZR + [0], False, 112)
    # enc1: input ch = [h0(0:16), h1(16:48)]; rows [h0, xA, xB, junk*14, h1]
    we1, b1 = _prep_weights(enc_w1, enc_b1, C0 + C1, C1,
                            list(range(16)) + ZR * 16 + list(range(16, 48)),
                            True, 128)
    # dec0: input ch = [e2(0:32), hd0(32:64)];
    # rows [hd1+ones+junk(0:32), e2A(32:64), hd0(64:96), e2B(96:128)]
    wd0a, bd0 = _prep_weights(dec_w0, dec_b0, C1 + C1, C1,
                              ZR * 32 + list(range(32)) +
                              list(range(32, 64)) + ZR * 32, True, 128)
    wd0b, _ = _prep_weights(dec_w0, dec_b0, C1 + C1, C1,
                            ZR * 32 + ZR * 32 +
                            list(range(32, 64)) + list(range(32)), True, 128)
    # dec1: input ch = [hd0(0:32), hd1(32:48)] — input is dec0's output seq
    wd1a, bd1 = _prep_weights(dec_w1, dec_b1, C1 + C0, C0,
                              list(range(32, 48)) + ZR * 16 + ZR * 32 +
                              list(range(32)) + ZR * 32, False, 112)
    wd1b = wd1a
    fcv = np.concatenate(
        [np.asarray(fc_w, np.float32).reshape(C0),
         np.asarray(fc_b, np.float32).reshape(1)]).reshape(17, 1)
    fcv = np.ascontiguousarray(fcv).astype(NP_BF16)

    in_maps = []
    for core in range(NCORES):
        xs = x[core * B:(core + 1) * B]      # [B, T, F]
        xp = np.zeros((T, B, SEG), np.float32)
        xp[:, :, 1:1 + F] = xs.transpose(1, 0, 2)
        in_maps.append({
            "x_pad": xp.astype(NP_BF16),
            "we0a": we0a, "we0b": we0b, "we1": we1,
            "wd0a": wd0a, "wd0b": wd0b, "wd1a": wd1a, "wd1b": wd1b,
            "fcv": fcv,
            "b0": b0, "b1": b1, "bd0": bd0, "bd1": bd1,
        })

    _CACHE["in_maps"] = in_maps
    res = run_bass_kernel_spmd(nc, in_maps, core_ids=list(range(NCORES)))
    outs = [res.results[i]["out"] for i in range(NCORES)]
    return np.concatenate(outs, axis=0).astype(np.float32)


if __name__ == "__main__":
    rng = np.random.default_rng(0)
    inputs = {
        "x": rng.standard_normal((B_TOT, T, F), dtype=np.float32),
        "enc_w0": rng.standard_normal((4 * C0, 1 + C0, 3, 3), dtype=np.float32) * 0.05,
        "enc_b0": np.zeros(4 * C0, np.float32),
        "enc_w1": rng.standard_normal((4 * C1, C0 + C1, 3, 3), dtype=np.float32) * 0.05,
        "enc_b1": np.zeros(4 * C1, np.float32),
        "dec_w0": rng.standard_normal((4 * C1, C1 + C1, 3, 3), dtype=np.float32) * 0.05,
        "dec_b0": np.zeros(4 * C1, np.float32),
        "dec_w1": rng.standard_normal((4 * C0, C1 + C0, 3, 3), dtype=np.float32) * 0.05,
        "dec_b1": np.zeros(4 * C0, np.float32),
        "fc_w": rng.standard_normal((1, C0, 1, 1), dtype=np.float32) * 0.05,
        "fc_b": np.zeros(1, np.float32),
    }
    out = kernel(**inputs)
    print("out", out.shape, out.dtype, np.abs(out).max())


# revision 18
# speedup vs baseline: 1.1598x; 1.0020x over previous
import sys

sys.path.insert(0, "/opt/trn_rl_repo")

import numpy as np
import ml_dtypes

import concourse.bass as bass
from concourse import bacc
import concourse.mybir as mybir
import concourse.tile as tile
from concourse.bass_utils import run_bass_kernel_spmd

# Problem constants (nn_ConvLSTMAutoencoder: B=128, T=100, F=64, hid [16,32])
B_TOT, T, F = 128, 100, 64
NCORES = 8
B = B_TOT // NCORES          # 16 batch per core (pure data parallelism)
SEG = F + 2                  # spatial row with 1 zero pad col each side
C0, C1 = 16, 32

F32 = mybir.dt.float32
BF16 = mybir.dt.bfloat16
NP_BF16 = ml_dtypes.bfloat16

ACT_DT = BF16                # arena / gate tensors / matmul inputs
C_DT = BF16                  # cell-state dtype (flip to F32 if accuracy needs)

Tanh = mybir.ActivationFunctionType.Tanh
Sigmoid = mybir.ActivationFunctionType.Sigmoid
MULT = mybir.AluOpType.mult
ADD = mybir.AluOpType.add

NB = 8                       # batches per matmul (8*64 = 512 = psum bank cap)

# Gate column spread along matmul M for every layer:
#   [i @ 0:C | f @ 32:32+C | o @ 64:64+C | g @ 96:96+C]
# Cell-state tiles keep c at rows 32:32+C (v/add operate at base 32).
# tanh(c) is written at rows 64:64+C to meet sigma(o) for the h product.


def _taps(nc, zt, wt, rhs_rows, arena):
    """3-tap conv along F as PSUM-accumulated matmuls, nb-major so each
    batch-half's z finishes as early as possible. wt: [K, 3, M]."""
    for nb in range(0, B, NB):
        for d in range(3):
            nc.tensor.matmul(
                zt[:, nb:nb + NB, :],
                wt[:, d, :],
                arena[rhs_rows, nb:nb + NB, d:d + F],
                start=(d == 0),
                stop=(d == 2),
            )


def _cell_sig(nc, wpool, z, C, bvec, ctile, h_of):
    """LSTM cell, act-heavy variant: true Sigmoid act for i/f/o rows plus a
    direct Tanh act for g (re-based to partition 0). Batch-halved."""
    s = wpool.tile([96, B, F], ACT_DT, tag="s", name="s")
    tg = wpool.tile([C, B, F], ACT_DT, tag="tg", name="tg")
    u = wpool.tile([32 + C, B, F], ACT_DT, tag="u", name="u")
    v = wpool.tile([32 + C, B, F], C_DT, tag="v", name="v")
    tc = wpool.tile([64 + C, B, F], ACT_DT, tag="tc", name="tc")
    halves = [(n0, n0 + NB) for n0 in range(0, B, NB)]
    for n0, n1 in halves:
        nc.scalar.activation(tg[:, n0:n1], z[96:96 + C, n0:n1], Tanh,
                             bias=bvec[96:96 + C])
        nc.scalar.activation(s[0:64 + C, n0:n1], z[0:64 + C, n0:n1], Sigmoid,
                             bias=bvec[0:64 + C])
        nc.vector.tensor_tensor(v[32:32 + C, n0:n1], s[32:32 + C, n0:n1],
                                ctile[32:32 + C, n0:n1], MULT)
        nc.vector.tensor_tensor(u[32:32 + C, n0:n1], s[0:C, n0:n1],
                                tg[:, n0:n1], MULT)
        nc.vector.tensor_tensor(ctile[32:32 + C, n0:n1], u[32:32 + C, n0:n1],
                                v[32:32 + C, n0:n1], ADD)
    for n0, n1 in halves:
        nc.scalar.activation(tc[64:64 + C, n0:n1], ctile[32:32 + C, n0:n1],
                             Tanh)
    for n0, n1 in halves:
        nc.vector.tensor_tensor(h_of(n0, n1), s[64:64 + C, n0:n1],
                                tc[64:64 + C, n0:n1], MULT)


def _cell_tanh(nc, wpool, z, C, M, bvec, ctile, h_of):
    """LSTM cell, DVE-heavy variant: one Tanh act over all gates (i/f/o rows
    pre-halved in weights), sigma fixup + g re-base on the vector engine."""
    th = wpool.tile([M, B, F], ACT_DT, tag="s", name="th")
    tg = wpool.tile([C, B, F], ACT_DT, tag="tg", name="tg")
    u = wpool.tile([32 + C, B, F], ACT_DT, tag="u", name="u")
    v = wpool.tile([32 + C, B, F], C_DT, tag="v", name="v")
    tc = wpool.tile([64 + C, B, F], ACT_DT, tag="tc", name="tc")
    halves = [(n0, n0 + NB) for n0 in range(0, B, NB)]
    for n0, n1 in halves:
        nc.scalar.activation(th[:, n0:n1], z[0:M, n0:n1], Tanh, bias=bvec[0:M])
        nc.vector.tensor_scalar(tg[:, n0:n1], th[96:96 + C, n0:n1],
                                1.0, 0.0, MULT, ADD)
        nc.vector.tensor_scalar(th[0:64 + C, n0:n1], th[0:64 + C, n0:n1],
                                0.5, 0.5, MULT, ADD)
        nc.vector.tensor_tensor(v[32:32 + C, n0:n1], th[32:32 + C, n0:n1],
                                ctile[32:32 + C, n0:n1], MULT)
        nc.vector.tensor_tensor(u[32:32 + C, n0:n1], th[0:C, n0:n1],
                                tg[:, n0:n1], MULT)
        nc.vector.tensor_tensor(ctile[32:32 + C, n0:n1], u[32:32 + C, n0:n1],
                                v[32:32 + C, n0:n1], ADD)
    for n0, n1 in halves:
        nc.scalar.activation(tc[64:64 + C, n0:n1], ctile[32:32 + C, n0:n1],
                             Tanh)
    for n0, n1 in halves:
        nc.vector.tensor_tensor(h_of(n0, n1), th[64:64 + C, n0:n1],
                                tc[64:64 + C, n0:n1], MULT)


def build_program():
    nc = bacc.Bacc(None)

    x_pad = nc.declare_dram_parameter("x_pad", [T, B, SEG], ACT_DT, isOutput=False)
    # enc0 lhsT variants A/B: K=18 rows [h0(16), xA, xB]
    we0a = nc.declare_dram_parameter("we0a", [18, 3, 112], ACT_DT, isOutput=False)
    we0b = nc.declare_dram_parameter("we0b", [18, 3, 112], ACT_DT, isOutput=False)
    we1 = nc.declare_dram_parameter("we1", [64, 3, 128], ACT_DT, isOutput=False)
    wd0a = nc.declare_dram_parameter("wd0a", [128, 3, 128], ACT_DT, isOutput=False)
    wd0b = nc.declare_dram_parameter("wd0b", [128, 3, 128], ACT_DT, isOutput=False)
    wd1a = nc.declare_dram_parameter("wd1a", [128, 3, 112], ACT_DT, isOutput=False)
    wd1b = nc.declare_dram_parameter("wd1b", [128, 3, 112], ACT_DT, isOutput=False)
    fcv = nc.declare_dram_parameter("fcv", [17, 1], ACT_DT, isOutput=False)
    sv = nc.declare_dram_parameter("sv", [128, 1], F32, isOutput=False)
    b0 = nc.declare_dram_parameter("b0", [112, 1], F32, isOutput=False)
    b1 = nc.declare_dram_parameter("b1", [128, 1], F32, isOutput=False)
    bd0 = nc.declare_dram_parameter("bd0", [128, 1], F32, isOutput=False)
    bd1 = nc.declare_dram_parameter("bd1", [112, 1], F32, isOutput=False)
    out = nc.declare_dram_parameter("out", [B, T, F], F32, isOutput=True)

    with tile.TileContext(nc) as tc:
        with (
            tc.tile_pool(name="const", bufs=1) as cpool,
            tc.tile_pool(name="state", bufs=1) as spool,
            tc.tile_pool(name="work", bufs=2) as wpool,
            tc.tile_pool(name="zp", bufs=2, space="PSUM") as zpool,
            tc.tile_pool(name="fcp", bufs=1, space="PSUM") as fcpool,
        ):
            we0at = cpool.tile([18, 3, 112], ACT_DT)
            we0bt = cpool.tile([18, 3, 112], ACT_DT)
            we1t = cpool.tile([64, 3, 128], ACT_DT)
            wd0at = cpool.tile([128, 3, 128], ACT_DT)
            wd0bt = cpool.tile([128, 3, 128], ACT_DT)
            wd1at = cpool.tile([128, 3, 112], ACT_DT)
            wd1bt = cpool.tile([128, 3, 112], ACT_DT)
            fcvt = cpool.tile([17, 1], ACT_DT)
            svt = cpool.tile([128, 1], F32)
            b0t = cpool.tile([112, 1], F32)
            b1t = cpool.tile([128, 1], F32)
            bd0t = cpool.tile([128, 1], F32)
            bd1t = cpool.tile([112, 1], F32)
            for dst, dsrc in [(we0at, we0a), (we0bt, we0b), (we1t, we1),
                              (wd0at, wd0a), (wd0bt, wd0b), (wd1at, wd1a),
                              (wd1bt, wd1b), (fcvt, fcv), (svt, sv), (b0t, b0),
                              (b1t, b1), (bd0t, bd0), (bd1t, bd1)]:
                nc.sync.dma_start(dst[:], dsrc[:])

            # Encoder arena rows: 0:16 h0 | 16 xA | 17 xB | 18:32 junk | 32:64 h1
            # Decoder arena rows: 0:16 hd1 | 16 ones | 17:32 junk |
            #                     32:64 e2A | 64:96 hd0 | 96:128 e2B
            arena_e = spool.tile([64, B, SEG], ACT_DT)
            arena_d = spool.tile([128, B, SEG], ACT_DT)
            seq = spool.tile([128, (T + 3) // 4, B, F], ACT_DT)
            nc.vector.memset(arena_e[:], 0.0)
            nc.vector.memset(arena_d[:], 0.0)
            nc.vector.memset(arena_d[0:17], 1.0)   # row 16 stays 1.0 (fc bias)
            nc.vector.memset(arena_d[0:16], 0.0)

            ce0 = spool.tile([32 + C0, B, F], C_DT)
            ce1 = spool.tile([32 + C1, B, F], C_DT)
            cd0 = spool.tile([32 + C1, B, F], C_DT)
            cd1 = spool.tile([32 + C0, B, F], C_DT)
            nc.vector.memset(ce0[32:32 + C0], 0.0)
            nc.vector.memset(ce1[32:32 + C1], 0.0)

            # ---------------- encoder ----------------
            nc.sync.dma_start(arena_e[16:17, :, :], x_pad[0:1, :, :])
            for t in range(T):
                if t + 1 < T:
                    xrow = 16 + ((t + 1) % 2)
                    nc.sync.dma_start(arena_e[xrow:xrow + 1, :, :],
                                      x_pad[t + 1:t + 2, :, :])

                z0 = zpool.tile([112, B, F], F32, tag="z", name="z0")
                _taps(nc, z0, we0at if t % 2 == 0 else we0bt,
                      slice(0, 18), arena_e)
                _cell_sig(nc, wpool, z0, C0, b0t, ce0,
                          lambda n0, n1: arena_e[0:C0, n0:n1, 1:1 + F])

                z1 = zpool.tile([128, B, F], F32, tag="z", name="z1")
                _taps(nc, z1, we1t, slice(0, 64), arena_e)
                _cell_tanh(nc, wpool, z1, C1, 128, b1t, ce1,
                           lambda n0, n1: arena_e[C1:2 * C1, n0:n1, 1:1 + F])

                r = (t % 4) * 32
                nc.sync.dma_start(seq[r:r + 32, t // 4],
                                  arena_e[C1:2 * C1, :, 1:1 + F])

            # ---------------- decoder init ----------------
            nc.sync.dma_start(arena_d[0:16, :, :], arena_e[0:16, :, :])
            nc.sync.dma_start(arena_d[64:96, :, :], arena_e[32:64, :, :])
            nc.sync.dma_start(cd0[32:32 + C1], ce1[32:32 + C1])
            nc.sync.dma_start(cd1[32:32 + C0], ce0[32:32 + C0])
            nc.sync.dma_start(arena_d[32:64, :, 1:1 + F], seq[0:32, 0])

            # ---------------- decoder ----------------
            # fc for step t is emitted at the top of iteration t+1 (hd1(t)
            # stays valid in the arena until dec1's h write late in t+1), so
            # its matmuls never block the next step's taps.
            def emit_fc(t):
                zfc = fcpool.tile([1, B, F], F32, tag="fc", name="zfc")
                for nb in range(0, B, NB):
                    nc.tensor.matmul(zfc[:, nb:nb + NB, :], fcvt[:],
                                     arena_d[0:17, nb:nb + NB, 1:1 + F],
                                     start=True, stop=True)
                ofc = wpool.tile([1, B, F], F32, tag="ofc", name="ofc")
                nc.scalar.copy(ofc[:], zfc[:])
                nc.sync.dma_start(out[:, t, :], ofc[0:1, :, :])

            for t in range(T):
                if t > 0:
                    emit_fc(t - 1)
                if t + 1 < T:
                    r = ((t + 1) % 4) * 32
                    e2rows = slice(32, 64) if (t + 1) % 2 == 0 else slice(96, 128)
                    nc.sync.dma_start(arena_d[e2rows, :, 1:1 + F],
                                      seq[r:r + 32, (t + 1) // 4])

                zd0 = zpool.tile([128, B, F], F32, tag="z", name="zd0")
                _taps(nc, zd0, wd0at if t % 2 == 0 else wd0bt,
                      slice(0, 128), arena_d)
                _cell_tanh(nc, wpool, zd0, C1, 128, bd0t, cd0,
                           lambda n0, n1: arena_d[64:96, n0:n1, 1:1 + F])

                zd1 = zpool.tile([112, B, F], F32, tag="z", name="zd1")
                _taps(nc, zd1, wd1at if t % 2 == 0 else wd1bt,
                      slice(0, 128), arena_d)
                _cell_sig(nc, wpool, zd1, C0, bd1t, cd1,
                          lambda n0, n1: arena_d[0:C0, n0:n1, 1:1 + F])

            emit_fc(T - 1)

    nc.finalize()
    return nc


def _prep_weights(w, b, Cin, C, row_map, halve_ifo, M):
    """[4C, Cin, 3, 3] -> lhsT [len(row_map), 3, M], bias [M, 1].

    Gate order i,f,o,g -> M columns i@0 f@32 o@64 g@96. row_map maps lhsT
    row -> input channel (-1 = zero row). halve_ifo scales i/f/o rows by
    0.5 for the tanh-trick cells.
    """
    w3 = np.asarray(w, np.float32).reshape(4 * C, Cin, 3, 3)[:, :, :, 1]
    b = np.asarray(b, np.float32).reshape(4 * C)
    lhsT = np.zeros((len(row_map), 3, M), np.float32)
    bvec = np.zeros((M, 1), np.float32)
    for gi, col0 in enumerate((0, 32, 64, 96)):
        scale = 0.5 if (halve_ifo and gi != 3) else 1.0
        for j in range(C):
            oc = gi * C + j
            bvec[col0 + j, 0] = b[oc] * scale
            for r, ch in enumerate(row_map):
                if ch >= 0:
                    lhsT[r, :, col0 + j] = w3[oc, ch, :] * scale
    return np.ascontiguousarray(lhsT).astype(NP_BF16), bvec


_CACHE = {}


def kernel(x, enc_w0, enc_b0, enc_w1, enc_b1, dec_w0, dec_b0, dec_w1, dec_b1,
           fc_w, fc_b):
    if "nc" not in _CACHE:
        _CACHE["nc"] = build_program()
    nc = _CACHE["nc"]

    x = np.asarray(x, np.float32)
    ZR = [-1]
    # enc0: input ch = [x(0), h0(1:17)]; arena rows [h0(16), xA, xB]
    we0a, b0 = _prep_weights(enc_w0, enc_b0, 1 + C0, C0,
                             list(range(1, 17)) + [0] + ZR, False, 112)
    sv = np.ones((128, 1), np.float32); sv[96:128] = 2.0
    we0b, _ = _prep_weights(enc_w0, enc_b0, 1 + C0, C0,
                            list(range(1, 17)) + ZR + [0], False, 112)
    # enc1: input ch = [h0(0:16), h1(16:48)]; rows [h0, xA, xB, junk*14, h1]
    we1, b1 = _prep_weights(enc_w1, enc_b1, C0 + C1, C1,
                            list(range(16)) + ZR * 16 + list(range(16, 48)),
                            True, 128)
    # dec0: input ch = [e2(0:32), hd0(32:64)];
    # rows [hd1+ones+junk(0:32), e2A(32:64), hd0(64:96), e2B(96:128)]
    wd0a, bd0 = _prep_weights(dec_w0, dec_b0, C1 + C1, C1,
                              ZR * 32 + list(range(32)) +
                              list(range(32, 64)) + ZR * 32, True, 128)
    wd0b, _ = _prep_weights(dec_w0, dec_b0, C1 + C1, C1,
                            ZR * 32 + ZR * 32 +
                            list(range(32, 64)) + list(range(32)), True, 128)
    # dec1: input ch = [hd0(0:32), hd1(32:48)] — input is dec0's output seq
    wd1a, bd1 = _prep_weights(dec_w1, dec_b1, C1 + C0, C0,
                              list(range(32, 48)) + ZR * 16 + ZR * 32 +
                              list(range(32)) + ZR * 32, False, 112)
    wd1b = wd1a
    fcv = np.concatenate(
        [np.asarray(fc_w, np.float32).reshape(C0),
         np.asarray(fc_b, np.float32).reshape(1)]).reshape(17, 1)
    fcv = np.ascontiguousarray(fcv).astype(NP_BF16)

    in_maps = []
    for core in range(NCORES):
        xs = x[core * B:(core + 1) * B]      # [B, T, F]
        xp = np.zeros((T, B, SEG), np.float32)
        xp[:, :, 1:1 + F] = xs.transpose(1, 0, 2)
        in_maps.append({
            "x_pad": xp.astype(NP_BF16),
            "we0a": we0a, "we0b": we0b, "we1": we1,
            "wd0a": wd0a, "wd0b": wd0b, "wd1a": wd1a, "wd1b": wd1b,
            "fcv": fcv, "sv": sv,
            "b0": b0, "b1": b1, "bd0": bd0, "bd1": bd1,
        })

    _CACHE["in_maps"] = in_maps
    res = run_bass_kernel_spmd(nc, in_maps, core_ids=list(range(NCORES)))
    outs = [res.results[i]["out"] for i in range(NCORES)]
    return np.concatenate(outs, axis=0).astype(np.float32)


if __name__ == "__main__":
    rng = np.random.default_rng(0)
    inputs = {
        "x": rng.standard_normal((B_TOT, T, F), dtype=np.float32),
        "enc_w0": rng.standard_normal((4 * C0, 1 + C0, 3, 3), dtype=np.float32) * 0.05,
        "enc_b0": np.zeros(4 * C0, np.float32),
        "enc_w1": rng.standard_normal((4 * C1, C0 + C1, 3, 3), dtype=np.float32) * 0.05,
        "enc_b1": np.zeros(4 * C1, np.float32),
        "dec_w0": rng.standard_normal((4 * C1, C1 + C1, 3, 3), dtype=np.float32) * 0.05,
        "dec_b0": np.zeros(4 * C1, np.float32),
        "dec_w1": rng.standard_normal((4 * C0, C1 + C0, 3, 3), dtype=np.float32) * 0.05,
        "dec_b1": np.zeros(4 * C0, np.float32),
        "fc_w": rng.standard_normal((1, C0, 1, 1), dtype=np.float32) * 0.05,
        "fc_b": np.zeros(1, np.float32),
    }
    out = kernel(**inputs)
    print("out", out.shape, out.dtype, np.abs(out).max())


# revision 20
# speedup vs baseline: 1.3914x; 1.1997x over previous
import sys

sys.path.insert(0, "/opt/trn_rl_repo")

import numpy as np
import ml_dtypes

import concourse.bass as bass
from concourse import bacc
import concourse.mybir as mybir
import concourse.tile as tile
from concourse.bass_utils import run_bass_kernel_spmd

# Problem constants (nn_ConvLSTMAutoencoder: B=128, T=100, F=64, hid [16,32])
B_TOT, T, F = 128, 100, 64
NCORES = 8
B = B_TOT // NCORES          # 16 batch per core (pure data parallelism)
SEG = F + 2                  # spatial row with 1 zero pad col each side
C0, C1 = 16, 32

F32 = mybir.dt.float32
BF16 = mybir.dt.bfloat16
NP_BF16 = ml_dtypes.bfloat16

ACT_DT = BF16                # arena / gate tensors / matmul inputs
C_DT = BF16                  # cell-state dtype (flip to F32 if accuracy needs)

Tanh = mybir.ActivationFunctionType.Tanh
Sigmoid = mybir.ActivationFunctionType.Sigmoid
MULT = mybir.AluOpType.mult
ADD = mybir.AluOpType.add

NB = 8                       # batches per matmul (8*64 = 512 = psum bank cap)

# Gate column spread along matmul M for every layer:
#   [i @ 0:C | f @ 32:32+C | o @ 64:64+C | g @ 96:96+C]
# Cell-state tiles keep c at rows 32:32+C (v/add operate at base 32).
# tanh(c) is written at rows 64:64+C to meet sigma(o) for the h product.


def _taps(nc, zt, wt, rhs_rows, arena):
    """3-tap conv along F as PSUM-accumulated matmuls, nb-major so each
    batch-half's z finishes as early as possible. wt: [K, 3, M]."""
    for nb in range(0, B, NB):
        for d in range(3):
            nc.tensor.matmul(
                zt[:, nb:nb + NB, :],
                wt[:, d, :],
                arena[rhs_rows, nb:nb + NB, d:d + F],
                start=(d == 0),
                stop=(d == 2),
            )


def _stages_sig(nc, wpool, z, C, bvec, ctile, h_of):
    """LSTM cell (act-heavy variant: Sigmoid + direct Tanh-g acts), split
    into stage emitters so two cells can interleave on the engine queues."""
    s = wpool.tile([96, B, F], ACT_DT, tag="s", name="s")
    tg = wpool.tile([C, B, F], ACT_DT, tag="tg", name="tg")
    u = wpool.tile([32 + C, B, F], ACT_DT, tag="u", name="u")
    v = wpool.tile([32 + C, B, F], C_DT, tag="v", name="v")
    tc = wpool.tile([64 + C, B, F], ACT_DT, tag="tc", name="tc")
    halves = [(n0, n0 + NB) for n0 in range(0, B, NB)]

    def acts():
        for n0, n1 in halves:
            nc.scalar.activation(tg[:, n0:n1], z[96:96 + C, n0:n1], Tanh,
                                 bias=bvec[96:96 + C])
            nc.scalar.activation(s[0:64 + C, n0:n1], z[0:64 + C, n0:n1],
                                 Sigmoid, bias=bvec[0:64 + C])

    def cupdate():
        for n0, n1 in halves:
            nc.vector.tensor_tensor(v[32:32 + C, n0:n1], s[32:32 + C, n0:n1],
                                    ctile[32:32 + C, n0:n1], MULT)
            nc.vector.tensor_tensor(u[32:32 + C, n0:n1], s[0:C, n0:n1],
                                    tg[:, n0:n1], MULT)
            nc.vector.tensor_tensor(ctile[32:32 + C, n0:n1],
                                    u[32:32 + C, n0:n1],
                                    v[32:32 + C, n0:n1], ADD)

    def actc():
        for n0, n1 in halves:
            nc.scalar.activation(tc[64:64 + C, n0:n1], ctile[32:32 + C, n0:n1],
                                 Tanh)

    def hmul():
        for n0, n1 in halves:
            nc.vector.tensor_tensor(h_of(n0, n1), s[64:64 + C, n0:n1],
                                    tc[64:64 + C, n0:n1], MULT)

    return acts, cupdate, actc, hmul


def _stages_tanh(nc, wpool, z, C, M, bvec, ctile, h_of):
    """LSTM cell (DVE-heavy variant: one Tanh act over all gates, i/f/o rows
    pre-halved in weights; sigma fixup + g re-base on the vector engine)."""
    th = wpool.tile([M, B, F], ACT_DT, tag="s", name="th")
    tg = wpool.tile([C, B, F], ACT_DT, tag="tg", name="tg")
    u = wpool.tile([32 + C, B, F], ACT_DT, tag="u", name="u")
    v = wpool.tile([32 + C, B, F], C_DT, tag="v", name="v")
    tc = wpool.tile([64 + C, B, F], ACT_DT, tag="tc", name="tc")
    halves = [(n0, n0 + NB) for n0 in range(0, B, NB)]

    def acts():
        for n0, n1 in halves:
            nc.scalar.activation(th[:, n0:n1], z[0:M, n0:n1], Tanh,
                                 bias=bvec[0:M])

    def cupdate():
        for n0, n1 in halves:
            nc.vector.tensor_scalar(tg[:, n0:n1], th[96:96 + C, n0:n1],
                                    1.0, 0.0, MULT, ADD)
            nc.vector.tensor_scalar(th[0:64 + C, n0:n1], th[0:64 + C, n0:n1],
                                    0.5, 0.5, MULT, ADD)
            nc.vector.tensor_tensor(v[32:32 + C, n0:n1], th[32:32 + C, n0:n1],
                                    ctile[32:32 + C, n0:n1], MULT)
            nc.vector.tensor_tensor(u[32:32 + C, n0:n1], th[0:C, n0:n1],
                                    tg[:, n0:n1], MULT)
            nc.vector.tensor_tensor(ctile[32:32 + C, n0:n1],
                                    u[32:32 + C, n0:n1],
                                    v[32:32 + C, n0:n1], ADD)

    def actc():
        for n0, n1 in halves:
            nc.scalar.activation(tc[64:64 + C, n0:n1], ctile[32:32 + C, n0:n1],
                                 Tanh)

    def hmul():
        for n0, n1 in halves:
            nc.vector.tensor_tensor(h_of(n0, n1), th[64:64 + C, n0:n1],
                                    tc[64:64 + C, n0:n1], MULT)

    return acts, cupdate, actc, hmul


def _interleave(cellA, cellB):
    """Emit two cells' stages in dependency-readiness order: A is the earlier
    cell (its z finishes first). Keeps every engine queue free of
    head-of-line blocking between the two in-flight cells."""
    aActs, aCup, aC, aH = cellA
    if cellB is None:
        aActs(); aCup(); aC(); aH()
        return
    bActs, bCup, bC, bH = cellB
    aActs()
    bActs()
    aCup()
    bCup()
    aC()
    aH()
    bC()
    bH()


def build_program():
    nc = bacc.Bacc(None)

    x_pad = nc.declare_dram_parameter("x_pad", [T, B, SEG], ACT_DT, isOutput=False)
    # enc0 lhsT variants A/B: K=18 rows [h0(16), xA, xB]
    we0a = nc.declare_dram_parameter("we0a", [18, 3, 112], ACT_DT, isOutput=False)
    we0b = nc.declare_dram_parameter("we0b", [18, 3, 112], ACT_DT, isOutput=False)
    we1 = nc.declare_dram_parameter("we1", [64, 3, 128], ACT_DT, isOutput=False)
    wd0a = nc.declare_dram_parameter("wd0a", [128, 3, 128], ACT_DT, isOutput=False)
    wd0b = nc.declare_dram_parameter("wd0b", [128, 3, 128], ACT_DT, isOutput=False)
    wd1a = nc.declare_dram_parameter("wd1a", [128, 3, 112], ACT_DT, isOutput=False)
    wd1b = nc.declare_dram_parameter("wd1b", [128, 3, 112], ACT_DT, isOutput=False)
    fcv = nc.declare_dram_parameter("fcv", [17, 1], ACT_DT, isOutput=False)
    sv = nc.declare_dram_parameter("sv", [128, 1], F32, isOutput=False)
    b0 = nc.declare_dram_parameter("b0", [112, 1], F32, isOutput=False)
    b1 = nc.declare_dram_parameter("b1", [128, 1], F32, isOutput=False)
    bd0 = nc.declare_dram_parameter("bd0", [128, 1], F32, isOutput=False)
    bd1 = nc.declare_dram_parameter("bd1", [112, 1], F32, isOutput=False)
    out = nc.declare_dram_parameter("out", [B, T, F], F32, isOutput=True)

    with tile.TileContext(nc) as tc:
        with (
            tc.tile_pool(name="const", bufs=1) as cpool,
            tc.tile_pool(name="state", bufs=1) as spool,
            tc.tile_pool(name="work", bufs=2) as wpool,
            tc.tile_pool(name="zp", bufs=2, space="PSUM") as zpool,
            tc.tile_pool(name="fcp", bufs=1, space="PSUM") as fcpool,
        ):
            we0at = cpool.tile([18, 3, 112], ACT_DT)
            we0bt = cpool.tile([18, 3, 112], ACT_DT)
            we1t = cpool.tile([64, 3, 128], ACT_DT)
            wd0at = cpool.tile([128, 3, 128], ACT_DT)
            wd0bt = cpool.tile([128, 3, 128], ACT_DT)
            wd1at = cpool.tile([128, 3, 112], ACT_DT)
            wd1bt = cpool.tile([128, 3, 112], ACT_DT)
            fcvt = cpool.tile([17, 1], ACT_DT)
            svt = cpool.tile([128, 1], F32)
            b0t = cpool.tile([112, 1], F32)
            b1t = cpool.tile([128, 1], F32)
            bd0t = cpool.tile([128, 1], F32)
            bd1t = cpool.tile([112, 1], F32)
            for dst, dsrc in [(we0at, we0a), (we0bt, we0b), (we1t, we1),
                              (wd0at, wd0a), (wd0bt, wd0b), (wd1at, wd1a),
                              (wd1bt, wd1b), (fcvt, fcv), (svt, sv), (b0t, b0),
                              (b1t, b1), (bd0t, bd0), (bd1t, bd1)]:
                nc.sync.dma_start(dst[:], dsrc[:])

            # Encoder arena rows: 0:16 h0 | 16 xA | 17 xB | 18:32 junk | 32:64 h1
            # Decoder arena rows: 0:16 hd1 | 16 ones | 17:32 junk |
            #                     32:64 e2A | 64:96 hd0 | 96:128 e2B
            arena_e = spool.tile([64, B, SEG], ACT_DT)
            arena_d = spool.tile([128, B, SEG], ACT_DT)
            seq = spool.tile([128, (T + 3) // 4, B, F], ACT_DT)
            nc.vector.memset(arena_e[:], 0.0)
            nc.vector.memset(arena_d[:], 0.0)
            nc.vector.memset(arena_d[0:17], 1.0)   # row 16 stays 1.0 (fc bias)
            nc.vector.memset(arena_d[0:16], 0.0)

            ce0 = spool.tile([32 + C0, B, F], C_DT)
            ce1 = spool.tile([32 + C1, B, F], C_DT)
            cd0 = spool.tile([32 + C1, B, F], C_DT)
            cd1 = spool.tile([32 + C0, B, F], C_DT)
            nc.vector.memset(ce0[32:32 + C0], 0.0)
            nc.vector.memset(ce1[32:32 + C1], 0.0)

            # ---------------- encoder ----------------
            # Software-pipelined: iteration t interleaves enc1(t) with
            # enc0(t+1) — the two cells in flight once h0(t) lands.
            nc.sync.dma_start(arena_e[16:17, :, :], x_pad[0:1, :, :])
            nc.sync.dma_start(arena_e[17:18, :, :], x_pad[1:2, :, :])
            z0 = zpool.tile([112, B, F], F32, tag="z", name="z0")
            _taps(nc, z0, we0at, slice(0, 18), arena_e)
            _interleave(_stages_sig(nc, wpool, z0, C0, b0t, ce0,
                        lambda n0, n1: arena_e[0:C0, n0:n1, 1:1 + F]), None)
            for t in range(T):
                if t + 2 < T:
                    xrow = 16 + ((t + 2) % 2)
                    nc.sync.dma_start(arena_e[xrow:xrow + 1, :, :],
                                      x_pad[t + 2:t + 3, :, :])
                z1 = zpool.tile([128, B, F], F32, tag="z", name="z1")
                _taps(nc, z1, we1t, slice(0, 64), arena_e)
                cellA = _stages_tanh(nc, wpool, z1, C1, 128, b1t, ce1,
                                     lambda n0, n1: arena_e[C1:2 * C1, n0:n1,
                                                            1:1 + F])
                cellB = None
                if t + 1 < T:
                    z0 = zpool.tile([112, B, F], F32, tag="z", name="z0")
                    _taps(nc, z0, we0at if (t + 1) % 2 == 0 else we0bt,
                          slice(0, 18), arena_e)
                    cellB = _stages_sig(nc, wpool, z0, C0, b0t, ce0,
                                        lambda n0, n1: arena_e[0:C0, n0:n1,
                                                               1:1 + F])
                _interleave(cellA, cellB)
                r = (t % 4) * 32
                nc.sync.dma_start(seq[r:r + 32, t // 4],
                                  arena_e[C1:2 * C1, :, 1:1 + F])

            # ---------------- decoder init ----------------
            nc.sync.dma_start(arena_d[0:16, :, :], arena_e[0:16, :, :])
            nc.sync.dma_start(arena_d[64:96, :, :], arena_e[32:64, :, :])
            nc.sync.dma_start(cd0[32:32 + C1], ce1[32:32 + C1])
            nc.sync.dma_start(cd1[32:32 + C0], ce0[32:32 + C0])
            nc.sync.dma_start(arena_d[32:64, :, 1:1 + F], seq[0:32, 0])

            # ---------------- decoder ----------------
            # Same pipelining: iteration t interleaves dec1(t) with
            # dec0(t+1). fc for step t-1 is emitted first (operands long
            # ready) so it never blocks the taps.
            def emit_fc(t):
                zfc = fcpool.tile([1, B, F], F32, tag="fc", name="zfc")
                for nb in range(0, B, NB):
                    nc.tensor.matmul(zfc[:, nb:nb + NB, :], fcvt[:],
                                     arena_d[0:17, nb:nb + NB, 1:1 + F],
                                     start=True, stop=True)
                ofc = wpool.tile([1, B, F], F32, tag="ofc", name="ofc")
                nc.scalar.copy(ofc[:], zfc[:])
                nc.sync.dma_start(out[:, t, :], ofc[0:1, :, :])

            zd0 = zpool.tile([128, B, F], F32, tag="z", name="zd0")
            _taps(nc, zd0, wd0at, slice(0, 128), arena_d)
            _interleave(_stages_tanh(nc, wpool, zd0, C1, 128, bd0t, cd0,
                        lambda n0, n1: arena_d[64:96, n0:n1, 1:1 + F]), None)
            for t in range(T):
                if t > 0:
                    emit_fc(t - 1)
                if t + 1 < T:
                    r = ((t + 1) % 4) * 32
                    e2rows = slice(32, 64) if (t + 1) % 2 == 0 else slice(96, 128)
                    nc.sync.dma_start(arena_d[e2rows, :, 1:1 + F],
                                      seq[r:r + 32, (t + 1) // 4])
                zd1 = zpool.tile([112, B, F], F32, tag="z", name="zd1")
                _taps(nc, zd1, wd1at if t % 2 == 0 else wd1bt,
                      slice(0, 128), arena_d)
                cellA = _stages_sig(nc, wpool, zd1, C0, bd1t, cd1,
                                    lambda n0, n1: arena_d[0:C0, n0:n1,
                                                           1:1 + F])
                cellB = None
                if t + 1 < T:
                    zd0 = zpool.tile([128, B, F], F32, tag="z", name="zd0")
                    _taps(nc, zd0, wd0at if (t + 1) % 2 == 0 else wd0bt,
                          slice(0, 128), arena_d)
                    cellB = _stages_tanh(nc, wpool, zd0, C1, 128, bd0t, cd0,
                                         lambda n0, n1: arena_d[64:96, n0:n1,
                                                                1:1 + F])
                _interleave(cellA, cellB)

            emit_fc(T - 1)

    nc.finalize()
    return nc


def _prep_weights(w, b, Cin, C, row_map, halve_ifo, M):
    """[4C, Cin, 3, 3] -> lhsT [len(row_map), 3, M], bias [M, 1].

    Gate order i,f,o,g -> M columns i@0 f@32 o@64 g@96. row_map maps lhsT
    row -> input channel (-1 = zero row). halve_ifo scales i/f/o rows by
    0.5 for the tanh-trick cells.
    """
    w3 = np.asarray(w, np.float32).reshape(4 * C, Cin, 3, 3)[:, :, :, 1]
    b = np.asarray(b, np.float32).reshape(4 * C)
    lhsT = np.zeros((len(row_map), 3, M), np.float32)
    bvec = np.zeros((M, 1), np.float32)
    for gi, col0 in enumerate((0, 32, 64, 96)):
        scale = 0.5 if (halve_ifo and gi != 3) else 1.0
        for j in range(C):
            oc = gi * C + j
            bvec[col0 + j, 0] = b[oc] * scale
            for r, ch in enumerate(row_map):
                if ch >= 0:
                    lhsT[r, :, col0 + j] = w3[oc, ch, :] * scale
    return np.ascontiguousarray(lhsT).astype(NP_BF16), bvec


_CACHE = {}


def kernel(x, enc_w0, enc_b0, enc_w1, enc_b1, dec_w0, dec_b0, dec_w1, dec_b1,
           fc_w, fc_b):
    if "nc" not in _CACHE:
        _CACHE["nc"] = build_program()
    nc = _CACHE["nc"]

    x = np.asarray(x, np.float32)
    ZR = [-1]
    # enc0: input ch = [x(0), h0(1:17)]; arena rows [h0(16), xA, xB]
    we0a, b0 = _prep_weights(enc_w0, enc_b0, 1 + C0, C0,
                             list(range(1, 17)) + [0] + ZR, False, 112)
    sv = np.ones((128, 1), np.float32); sv[96:128] = 2.0
    we0b, _ = _prep_weights(enc_w0, enc_b0, 1 + C0, C0,
                            list(range(1, 17)) + ZR + [0], False, 112)
    # enc1: input ch = [h0(0:16), h1(16:48)]; rows [h0, xA, xB, junk*14, h1]
    we1, b1 = _prep_weights(enc_w1, enc_b1, C0 + C1, C1,
                            list(range(16)) + ZR * 16 + list(range(16, 48)),
                            True, 128)
    # dec0: input ch = [e2(0:32), hd0(32:64)];
    # rows [hd1+ones+junk(0:32), e2A(32:64), hd0(64:96), e2B(96:128)]
    wd0a, bd0 = _prep_weights(dec_w0, dec_b0, C1 + C1, C1,
                              ZR * 32 + list(range(32)) +
                              list(range(32, 64)) + ZR * 32, True, 128)
    wd0b, _ = _prep_weights(dec_w0, dec_b0, C1 + C1, C1,
                            ZR * 32 + ZR * 32 +
                            list(range(32, 64)) + list(range(32)), True, 128)
    # dec1: input ch = [hd0(0:32), hd1(32:48)] — input is dec0's output seq
    wd1a, bd1 = _prep_weights(dec_w1, dec_b1, C1 + C0, C0,
                              list(range(32, 48)) + ZR * 16 + ZR * 32 +
                              list(range(32)) + ZR * 32, False, 112)
    wd1b = wd1a
    fcv = np.concatenate(
        [np.asarray(fc_w, np.float32).reshape(C0),
         np.asarray(fc_b, np.float32).reshape(1)]).reshape(17, 1)
    fcv = np.ascontiguousarray(fcv).astype(NP_BF16)

    in_maps = []
    for core in range(NCORES):
        xs = x[core * B:(core + 1) * B]      # [B, T, F]
        xp = np.zeros((T, B, SEG), np.float32)
        xp[:, :, 1:1 + F] = xs.transpose(1, 0, 2)
        in_maps.append({
            "x_pad": xp.astype(NP_BF16),
            "we0a": we0a, "we0b": we0b, "we1": we1,
            "wd0a": wd0a, "wd0b": wd0b, "wd1a": wd1a, "wd1b": wd1b,
            "fcv": fcv, "sv": sv,
            "b0": b0, "b1": b1, "bd0": bd0, "bd1": bd1,
        })

    _CACHE["in_maps"] = in_maps
    res = run_bass_kernel_spmd(nc, in_maps, core_ids=list(range(NCORES)))
    outs = [res.results[i]["out"] for i in range(NCORES)]
    return np.concatenate(outs, axis=0).astype(np.float32)


if __name__ == "__main__":
    rng = np.random.default_rng(0)
    inputs = {
        "x": rng.standard_normal((B_TOT, T, F), dtype=np.float32),
        "enc_w0": rng.standard_normal((4 * C0, 1 + C0, 3, 3), dtype=np.float32) * 0.05,
        "enc_b0": np.zeros(4 * C0, np.float32),
        "enc_w1": rng.standard_normal((4 * C1, C0 + C1, 3, 3), dtype=np.float32) * 0.05,
        "enc_b1": np.zeros(4 * C1, np.float32),
        "dec_w0": rng.standard_normal((4 * C1, C1 + C1, 3, 3), dtype=np.float32) * 0.05,
        "dec_b0": np.zeros(4 * C1, np.float32),
        "dec_w1": rng.standard_normal((4 * C0, C1 + C0, 3, 3), dtype=np.float32) * 0.05,
        "dec_b1": np.zeros(4 * C0, np.float32),
        "fc_w": rng.standard_normal((1, C0, 1, 1), dtype=np.float32) * 0.05,
        "fc_b": np.zeros(1, np.float32),
    }
    out = kernel(**inputs)
    print("out", out.shape, out.dtype, np.abs(out).max())
